# revision 23
# baseline (speedup 1.0000x reference)
"""Multi-head attention (B=2, S=2048, D=768, H=16, HD=48) on 8 trn2 NeuronCores.

Sharding: data-parallel over batch (2) x tensor-parallel over head groups (4).
Core c handles batch b=c//4, heads 4g..4g+3 where g=c%4. Each core returns the
partial output sum over its 4 heads; the host adds the 4 group partials per batch.

Per-core algorithm (all matmuls fp32r = full-rate TF32-like):
  xT = transpose(x) via PE                                  [768, 2048]
  qT/kT = (Wqk padded to 64 rows/head).T-projection         [128, pair, 2048]
          (pair tile: head 2p at partitions 0:48, head 2p+1 at 64:112, zero pad)
  v     = x @ Wv.T in 72-stride head layout + ones col 64   [128, 16, 288]
  per (head pair, sq-stripe of 512):
    for sk-chunk c in 16:
      scoresT(both heads) -> one [128, 1024] psum tile; the two K=64 strip
      matmuls use disjoint PE row groups and run concurrently
      E^T = exp(scoresT / sqrt(48)) (single ACT instr, psum->sbuf, fp32r)
      uT_h += vaug_h.T @ E^T_h      (ones col 64 makes uT row 64 = denom)
    rden = 1/uT[64] ; broadcast via DRAM bounce ; ctxT_h = uT[0:48] * rden
  out_partial = ctxT.T @ Wo.T (zero-padded Wo rows kill the pad partitions)

Schedule (single fused pipeline, PSUM shared by all stages through two
tag-compatible pools = exactly 8 banks):
  - the first attention stripe is woven into the x-load/transpose/QKV prefix
    (chunk c only needs the kT column group c//4, produced in the same group),
    so the ACT exp stream starts ~15us in instead of after the whole prefix;
  - within every stripe, scores for chunk c+1 issue before ctx of chunk c
    (software pipeline) so the PE never sits behind the exp latency;
  - output-projection tiles run one stripe behind pair-1's attention so they
    never wait on a freshly-written ctxT epilogue;
  - the last stripe's epilogue uses a rank-1 PE broadcast (ones x rden)
    instead of the DRAM bounce to shorten the tail dependency chain.
Steady state is ACT-bound at ~1.2us/chunk (exp 1.03us + dispatch), the
hard floor for softmax on one ScalarE. Cost model: ~226us/core.
"""
import numpy as np

DIM = 768
HEADS_PER_CORE = 4
HD = 48
B, S = 2, 2048
SCALE = HD ** -0.5
N_CORES = 8
KCH = 6          # 768 / 128 contraction chunks
SQT = S // 128   # 16 s-tiles of 128
NST = 4          # sq stripes of 512
CKS = S // 128   # 16 sk chunks


def build_program(iters: int = 1):
    import concourse.bass as bass
    import concourse.bacc as bacc
    import concourse.mybir as mybir
    import concourse.tile as tile
    from concourse.masks import make_identity

    f32 = mybir.dt.float32
    f32r = mybir.dt.float32r
    AF = mybir.ActivationFunctionType
    MUL = mybir.AluOpType.mult
    ADD = mybir.AluOpType.add

    nc = bacc.Bacc("TRN2", target_bir_lowering=False, debug=False,
                   num_devices=N_CORES)
    x_d = nc.dram_tensor("x", [S, DIM], f32, kind="ExternalInput").ap()
    wq_d = nc.dram_tensor("wqT", [DIM, 256], f32r, kind="ExternalInput").ap()
    wk_d = nc.dram_tensor("wkT", [DIM, 256], f32r, kind="ExternalInput").ap()
    wv_d = nc.dram_tensor("wvT", [DIM, 288], f32r, kind="ExternalInput").ap()
    wo_d = nc.dram_tensor("woT", [2, 128, DIM], f32r, kind="ExternalInput").ap()
    out_d = nc.dram_tensor("out", [S, DIM], f32, kind="ExternalOutput").ap()

    x_r = x_d.rearrange("(u p) d -> u p d", p=128)   # 4 super-tiles of 4 s-tiles
    out_r = out_d.rearrange("(t p) d -> t p d", p=128)

    with tile.TileContext(nc) as tc:
        with tc.tile_pool(name="const", bufs=1) as const_pool:
            ident = const_pool.tile([128, 128], f32, tag="ident")
            make_identity(nc, ident[:])

            for _ in range(iters):
                _emit_pass(nc, tc, bass, mybir, f32, f32r, AF, MUL, ADD,
                           ident, x_r, wq_d, wk_d, wv_d, wo_d, out_r)

    nc.compile()
    return nc


def _emit_pass(nc, tc, bass, mybir, f32, f32r, AF, MUL, ADD,
               ident, x_r, wq_d, wk_d, wv_d, wo_d, out_r):
    with tc.tile_pool(name="persist", bufs=1) as persist:
        xT = persist.tile([128, KCH, S], f32r, tag="xt")
        wq_sb = persist.tile([128, KCH, 256], f32r, tag="wq")
        wk_sb = persist.tile([128, KCH, 256], f32r, tag="wk")
        wv_sb = persist.tile([128, KCH, 288], f32r, tag="wv")
        wo_sb = persist.tile([128, 2, DIM], f32r, tag="wo")

        qT = persist.tile([128, 2, S], f32r, tag="qt")
        kT = persist.tile([128, 2, S], f32r, tag="kt")
        vaug = persist.tile([128, CKS, 288], f32r, tag="vaug")
        vaug4 = vaug[:].rearrange("p t (h e) -> p t h e", e=72)
        xT2 = xT[:].rearrange("p (k h) s -> p k h s", h=2)

        # ---- Fused pipeline ----
        # PSUM is shared by ALL stages through two tag-compatible pools
        # (2 x 4KB "big" slots + 4 x 2KB "small" slots = exactly 8 banks), so
        # the first attention stripe interleaves into the projection phase
        # (ACT exp starts ~15us in instead of ~60us) and the output projection
        # interleaves into the second pair's attention stripes.
        ctxT = persist.tile([128, 2, S], f32r, tag="ctxt")
        nc.vector.memset(ctxT[:].bitcast(f32), 0.0)
        ones_sb = persist.tile([1, 64], f32r, tag="ones")
        nc.vector.memset(ones_sb[:].bitcast(f32), 1.0)

        with tc.tile_pool(name="xin", bufs=3) as xpool, \
             tc.tile_pool(name="et", bufs=4) as e_pool, \
             tc.tile_pool(name="small", bufs=2) as small, \
             tc.tile_pool(name="dramp", bufs=2, space="DRAM") as dram_pool, \
             tc.tile_pool(name="osb", bufs=3) as out_pool, \
             tc.tile_pool(name="psBIG", bufs=2, space="PSUM") as ps_big, \
             tc.tile_pool(name="psSMALL", bufs=4, space="PSUM") as ps_small:

            def qk_group(w_sb, dest, p, n):
                ps = ps_big.tile([128, 512], f32, tag="big",
                                 name=f"qk_{id(w_sb) % 97}_{p}_{n}")
                for kk in range(KCH):
                    nc.tensor.matmul(
                        ps[:], w_sb[:, kk, bass.ts(p, 128)],
                        xT[:, kk, bass.ts(n, 512)],
                        start=(kk == 0), stop=(kk == KCH - 1))
                nc.vector.tensor_copy(dest[:, p, bass.ts(n, 512)], ps[:])

            def v_group(t):
                ps = ps_big.tile([128, 288], f32, tag="big", name=f"v_{t}")
                for kk in range(KCH):
                    nc.tensor.matmul(
                        ps[:], xT[:, kk, bass.ts(t, 128)], wv_sb[:, kk, :],
                        start=(kk == 0), stop=(kk == KCH - 1))
                nc.vector.tensor_copy(vaug[:, t, :], ps[:])
                nc.vector.memset(vaug4[:, t, :, 64:65].bitcast(f32), 1.0)

            def alloc_uts(p, s):
                return [ps_small.tile([65, 512], f32, tag="small",
                                      name=f"ut_{p}_{s}_{j2}")
                        for j2 in range(2)]

            def scores_part(p, s, c):
                sc = ps_big.tile([128, 1024], f32, tag="big",
                                 name=f"sc_{p}_{s}_{c}")
                for j2 in range(2):
                    lo, hi = 64 * j2, 64 * j2 + 64
                    nc.tensor.matmul(
                        sc[:, bass.ts(j2, 512)],
                        kT[lo:hi, p, bass.ts(c, 128)],
                        qT[lo:hi, p, bass.ts(s, 512)],
                        start=True, stop=True)
                e_t = e_pool.tile([128, 1024], f32r, tag="et",
                                  name=f"et_{p}_{s}_{c}")
                nc.scalar.activation(e_t[:], sc[:], AF.Exp, scale=SCALE)
                return e_t

            def ctx_part(p, s, uts, c, e_t):
                for j2 in range(2):
                    nc.tensor.matmul(
                        uts[j2][:], vaug4[:, c, 2 * p + j2, 0:65],
                        e_t[:, bass.ts(j2, 512)],
                        start=(c == 0), stop=(c == CKS - 1))

            def stripe_chunk(p, s, uts, c):
                e_t = scores_part(p, s, c)
                ctx_part(p, s, uts, c, e_t)

            def stripe_epilogue(p, s, uts, fast=False):
                for j2 in range(2):
                    rrep = small.tile([48, 512], f32, tag="rrep")
                    if fast:
                        # tail path: rank-1 PE broadcast (psum is free by now),
                        # ~1.5us chain instead of the ~5us double-DMA bounce
                        rden = small.tile([1, 512], f32r, tag="rdenr")
                        with nc.allow_low_precision(reason="f32r=f32 bits"):
                            nc.vector.reciprocal(rden[:], uts[j2][64:65, :])
                        rep_ps = ps_big.tile([128, 512], f32, tag="big",
                                             name=f"rep_{p}_{s}_{j2}")
                        nc.tensor.matmul(rep_ps[0:48, :], ones_sb[0:1, 0:48],
                                         rden[:], start=True, stop=True)
                        nc.vector.tensor_copy(rrep[:], rep_ps[0:48, :])
                    else:
                        rden = small.tile([1, 512], f32, tag="rden")
                        nc.vector.reciprocal(rden[:], uts[j2][64:65, :])
                        dsc = dram_pool.tile([1, 512], f32, tag="dsc")
                        nc.sync.dma_start(dsc[:], rden[:])
                        nc.sync.dma_start(rrep[:], dsc[0:1, :].to_broadcast((48, 512)))
                    nc.vector.tensor_tensor(
                        out=ctxT[64 * j2:64 * j2 + 48, p, bass.ts(s, 512)],
                        in0=uts[j2][0:48, :], in1=rrep[:], op=MUL)

            def d_tile(t):
                po = ps_big.tile([128, 1024], f32, tag="big", name=f"po_{t}")
                for c2 in range(2):
                    for nlo, nsz in ((0, 512), (512, 256)):
                        nc.tensor.matmul(
                            po[:, bass.ds(nlo, nsz)],
                            ctxT[:, c2, bass.ts(t, 128)],
                            wo_sb[:, c2, bass.ds(nlo, nsz)],
                            start=(c2 == 0), stop=(c2 == 1))
                ob = out_pool.tile([128, DIM], f32, tag="ob")
                nc.vector.tensor_copy(ob[:], po[:, 0:DIM])
                nc.sync.dma_start(out_r[t], ob[:])

            # prefix: x-load + transpose + QKV, with (p0, s0) chunks woven in
            # (software-pipelined across n-groups via e_prev)
            uts00 = alloc_uts(0, 0)
            e_prev = None
            for n in range(NST):
                for tq in range(4):
                    t = 4 * n + tq
                    x_sb = xpool.tile([128, DIM], f32, tag="xsb", name=f"xsb_{t}")
                    nc.sync.dma_start(x_sb[:], x_r[t])
                    for j2 in range(KCH // 2):
                        pt = ps_small.tile([128, 256], f32, tag="small",
                                           name=f"tr_{t}_{j2}")
                        nc.tensor.transpose(
                            pt[:, 0:128], x_sb[:, bass.ts(2 * j2, 128)], ident[:])
                        nc.tensor.transpose(
                            pt[:, 128:256], x_sb[:, bass.ts(2 * j2 + 1, 128)], ident[:])
                        nc.vector.tensor_copy(
                            xT2[:, j2, :, bass.ts(t, 128)],
                            pt[:].rearrange("p (h s) -> p h s", h=2))
                if n == 0:
                    nc.sync.dma_start(wk_sb[:], wk_d.rearrange("(k p) e -> p k e", p=128))
                    nc.sync.dma_start(wq_sb[:], wq_d.rearrange("(k p) e -> p k e", p=128))
                    nc.sync.dma_start(wv_sb[:], wv_d.rearrange("(k p) e -> p k e", p=128))
                    nc.sync.dma_start(wo_sb[:], wo_d.rearrange("c p d -> p c d"))
                for p in range(2):
                    qk_group(wk_sb, kT, p, n)
                for p in range(2):
                    qk_group(wq_sb, qT, p, n)
                for tq in range(4):
                    v_group(4 * n + tq)
                for cq in range(4):
                    c = 4 * n + cq
                    e_t = scores_part(0, 0, c)
                    if e_prev is not None:
                        ctx_part(0, 0, uts00, c - 1, e_prev)
                    e_prev = e_t
            ctx_part(0, 0, uts00, CKS - 1, e_prev)
            stripe_epilogue(0, 0, uts00)

            # remaining stripes of pair 0 (software-pipelined: scores of
            # chunk c+1 issue before ctx of chunk c, hiding the exp latency)
            def pipelined_stripe(p, s, interleave=None, fast_epi=False):
                uts = alloc_uts(p, s)
                e_prev = scores_part(p, s, 0)
                for c in range(1, CKS):
                    e_t = scores_part(p, s, c)
                    ctx_part(p, s, uts, c - 1, e_prev)
                    e_prev = e_t
                    if interleave is not None:
                        interleave(c)
                ctx_part(p, s, uts, CKS - 1, e_prev)
                stripe_epilogue(p, s, uts, fast=fast_epi)

            for s in range(1, NST):
                pipelined_stripe(0, s)

            # pair 1, with output-projection tiles woven in one stripe behind
            # (D tiles for stripe s-1 run while stripe s computes, so they never
            # wait on the freshly-written ctxT epilogue chain)
            for s in range(NST):
                def ilv(c, s=s):
                    if s >= 1 and c in (3, 6, 9, 12):
                        d_tile(4 * (s - 1) + (c - 3) // 3)
                pipelined_stripe(1, s, interleave=ilv,
                                 fast_epi=(s == NST - 1))
            for tq in range(4):
                d_tile(4 * (NST - 1) + tq)


def shard_inputs(x, Wq, Wk, Wv, Wo):
    """Host-side prep: per-core input dicts with padded transposed weights."""
    in_maps = []
    for c in range(N_CORES):
        b, g = c // 4, c % 4
        wqT = np.zeros((DIM, 256), np.float32)
        wkT = np.zeros((DIM, 256), np.float32)
        wvT = np.zeros((DIM, 288), np.float32)
        woT = np.zeros((2, 128, DIM), np.float32)
        for j in range(HEADS_PER_CORE):
            h = HEADS_PER_CORE * g + j
            rows = slice(HD * h, HD * h + HD)
            wqT[:, 64 * j:64 * j + HD] = Wq[rows, :].T
            wkT[:, 64 * j:64 * j + HD] = Wk[rows, :].T
            wvT[:, 72 * j:72 * j + HD] = Wv[rows, :].T
            woT[j // 2, 64 * (j % 2):64 * (j % 2) + HD, :] = Wo[:, rows].T
        in_maps.append({
            "x": np.ascontiguousarray(x[b], np.float32),
            "wqT": wqT, "wkT": wkT, "wvT": wvT, "woT": woT,
        })
    return in_maps


_CACHED_NC = None


def kernel(x, Wq, Wk, Wv, Wo):
    from concourse.bass_utils import run_bass_kernel_spmd

    global _CACHED_NC
    if _CACHED_NC is None:
        _CACHED_NC = build_program(iters=1)
    nc = _CACHED_NC

    x = np.asarray(x, np.float32)
    in_maps = shard_inputs(x, np.asarray(Wq, np.float32), np.asarray(Wk, np.float32),
                           np.asarray(Wv, np.float32), np.asarray(Wo, np.float32))
    res = run_bass_kernel_spmd(nc, in_maps, core_ids=list(range(N_CORES)))
    out = np.zeros((B, S, DIM), np.float32)
    for c in range(N_CORES):
        out[c // 4] += res.results[c]["out"]
    return out
